# revision 6
# baseline (speedup 1.0000x reference)
"""Trainium2 Bass kernel for nn_CrossAttention_28183575396415.

The reference block-mask gives every query exactly one key (kv = q_idx // 3),
so the softmax weight is identically 1 and the q/k projections, RMSNorm and
RoPE are dead code.  The module reduces to

    out[b, t] = x_kv[b, t // 3] @ Wv.T @ Wproj.T
              = x_kv[b, t // 3] @ WfT          with WfT = Wv.T @ Wproj.T

Strategy (8 NeuronCores, SPMD):
  - Host folds the two projection matrices into WfT (computed in float64)
    - constant folding of adjacent linear layers.
  - The 4*2048 = 8192 kv rows are row-sharded 8 ways (1024 rows/core).
    Each core's shard is pre-transposed on host so every device DMA is a
    natural contiguous load; the shard and the weight are concatenated into
    one [1024(k), 2048] bf16 input so each k-tile arrives in a single DMA:
        xw[:, :1024]  = x_shard.T   (k on partitions = contraction dim)
        xw[:, 1024:]  = WfT
  - Device: z = xT.T @ WfT, K accumulated in PSUM over 8 k-tiles.
    Column half 0 runs k-major (overlapping the input stream), column half 1
    runs m-major so finished row tiles retire one at a time and the output
    DMA stream stays busy until the end instead of bursting at the tail.
  - Each z tile is written to HBM with a single DMA whose source AP repeats
    the tile 3x (stride-0 middle dim) - the t//3 replication - giving this
    core's contiguous [3072, 1024] slice of the flattened output in bf16.
  - Host unshard = concatenate the 8 slices and upcast to float32.
"""

import json
import os

import numpy as np

import concourse.bass as bass
import concourse.mybir as mybir
from bass_rust import AP
from concourse.tile import TileContext
from concourse.vector_clock import ScopedClock
from concourse.bass_utils import run_bass_kernel_spmd

P = 128          # partitions
C = 1024         # model dim
K_T = C // P     # k tiles
M_T = C // P     # row tiles per core shard
N = 512          # matmul free dim (one PSUM bank of fp32)
L = 3            # replication factor (Tq // Tkv)
ROWS_PER_CORE = 1024
N_CORES = 8

# compute dtype: "bf16" (half the input DMA), "f32r"/"f32" for debugging
COMPUTE_DT = os.environ.get("KERNEL_COMPUTE_DT", "bf16")
# output dtype on device: "bf16" (host upcasts) or "f32"
OUT_DT = os.environ.get("KERNEL_OUT_DT", "bf16")
# "bcast": one DMA per z tile with stride-0 replication; "multi": 3 DMAs
REP_MODE = os.environ.get("KERNEL_REP_MODE", "bcast")


class SlimTailTileContext(TileContext):
    """Tile's kernel tail is drain -> barrier -> ~280 serialized per-semaphore
    clear instructions -> barrier (~8 us measured).  The clears only matter if
    the loaded NEFF executes more than once; every kernel() call here builds a
    fresh jit executable (fresh NEFF load, semaphores re-initialized), so skip
    them and the second barrier.  The drain still waits for every DMA queue,
    so outputs are complete before the program ends."""

    def _drain_and_barrier(self, tick_clock, wait_clock):
        drain_inst = self.nc.sync.drain()
        wait_clock.add_sem_waits(
            drain_inst.ins, ScopedClock({None: tick_clock.global_clock})
        )
        popped = self.nc._tile_sem_poison_stack.pop()
        assert popped is self._sem_poison


def _split_multiwaits(nc: bass.Bass) -> None:
    """This container's walrus allows only ONE sync-wait on several
    instruction formats (Drain/CTRL, Matmult's LDWEIGHTS half, ...).  Tile
    can emit more.  Post-pass the serialized BIR: for any instruction with
    >1 on_wait, hoist all but the last wait onto single-wait EventSemaphore
    carriers inserted immediately before it on the same engine (waits then
    execute in queue order - semantics unchanged)."""
    raw = bass.Bass.to_json_bytes(nc)
    j = json.loads(raw)
    for f in j["functions"]:
        for bb in f["blocks"]:
            new_insts = []
            for ins in bb["instructions"]:
                si = ins.get("sync_info")
                waits = si.get("on_wait", []) if si else []
                if len(waits) > 1:
                    for i, w in enumerate(waits[:-1]):
                        carrier = {
                            "engine": ins["engine"],
                            "ins": [],
                            "outs": [],
                            "name": f"{ins['name']}_hw{i}",
                            "opcode": "EventSemaphore",
                            "sync_info": {"on_update": [], "on_wait": [w]},
                        }
                        if "debug" in ins:
                            carrier["debug"] = ins["debug"]
                        new_insts.append(carrier)
                    si["on_wait"] = waits[-1:]
                new_insts.append(ins)
            bb["instructions"] = new_insts
    patched = json.dumps(j).encode()
    nc.to_json_bytes = lambda: patched


def _rep3_src(zh_ap):
    """Source AP reading a [P, N] SBUF tile as [P, L, N] via a stride-0
    middle dim - the DMA replicates each row L times."""
    lay = zh_ap.ap
    assert len(lay) == 2, lay
    return AP(tensor=zh_ap.tensor, offset=zh_ap.offset, ap=[lay[0], [0, L], lay[1]])


def _build(compute_dt: str, out_dt: str, rep_mode: str) -> bass.Bass:
    nc = bass.Bass("TRN2")
    in_mydt = {
        "bf16": mybir.dt.bfloat16,
        "f32r": mybir.dt.float32r,
        "f32": mybir.dt.float32,
    }[compute_dt]
    out_mydt = {"bf16": mybir.dt.bfloat16, "f32": mybir.dt.float32}[out_dt]

    W2 = ROWS_PER_CORE + C  # concatenated [x | w] free dim
    xw = nc.dram_tensor("xw", [C, W2], in_mydt, kind="ExternalInput")
    out = nc.dram_tensor(
        "out", [L * ROWS_PER_CORE, C], out_mydt, kind="ExternalOutput"
    )
    # out row (L*g + r) <- z row g
    out_rep = out.rearrange("(g r) c -> g r c", r=L)  # [1024, L, 1024]

    with SlimTailTileContext(nc) as tc:
        with (
            tc.tile_pool(name="xw", bufs=1) as xw_pool,
            tc.tile_pool(name="wu", bufs=1) as wu_pool,
            tc.tile_pool(name="psum", bufs=8, space="PSUM") as psum_pool,
            tc.tile_pool(name="zout", bufs=8) as z_pool,
        ):
            # Warmup scratch: the PE starts clock-gated at K=4/8 (1.2 GHz)
            # and flips to 8/8 only after a ~3.4us sustained-activity window.
            # Dummy matmuls on REAL data (zeros don't register activity)
            # during the otherwise-idle input wait start that window early.
            wu = wu_pool.tile([P, 2 * P], in_mydt, name="wu", tag="wu")
            nc.sync.dma_start(wu[:], xw[:P, : 2 * P])

            # Input: one tile per k-tile, loaded as two half-DMAs - the x
            # columns on sync's HWDGE ring, the W columns on scalar's - so
            # both rings stream in parallel and each k-tile lands ~1.4us
            # apart.  k0 is further quartered so the first real matmul can
            # start as early as possible.
            xwk = []
            for k in range(K_T):
                t = xw_pool.tile([P, W2], in_mydt, name=f"xw{k}", tag=f"xw{k}")
                xwk.append(t)
                if k == 0:
                    nc.sync.dma_start(t[:, : N], xw[:P, : N])
                    nc.scalar.dma_start(
                        t[:, ROWS_PER_CORE : ROWS_PER_CORE + N],
                        xw[:P, ROWS_PER_CORE : ROWS_PER_CORE + N],
                    )
                    nc.sync.dma_start(
                        t[:, N : ROWS_PER_CORE], xw[:P, N : ROWS_PER_CORE]
                    )
                    nc.scalar.dma_start(
                        t[:, ROWS_PER_CORE + N :], xw[:P, ROWS_PER_CORE + N :]
                    )
                else:
                    nc.sync.dma_start(
                        t[:, : ROWS_PER_CORE], xw[k * P : (k + 1) * P, : ROWS_PER_CORE]
                    )
                    nc.scalar.dma_start(
                        t[:, ROWS_PER_CORE :], xw[k * P : (k + 1) * P, ROWS_PER_CORE :]
                    )

            out_eng = [nc.sync, nc.scalar]
            n_trig = [0]

            def store_row(zh, m, lo=0, hi=C, eng=None):
                """One DMA writing z rows [m*P,(m+1)*P) x cols [lo,hi) to all
                3 replicas - full rows give 2 KiB HBM runs, 3 adjacent."""
                dst = out_rep[m * P : (m + 1) * P, :, lo:hi]
                src = zh[:, lo:hi]
                if eng is None:
                    eng = out_eng[n_trig[0] % 2]
                    n_trig[0] += 1
                if rep_mode == "bcast":
                    eng.dma_start(dst, _rep3_src(src))
                else:
                    for r in range(L):
                        eng.dma_start(out_rep[m * P : (m + 1) * P, r, lo:hi], src)

            def evict_row(ps_cc0, ps_cc1, m, last=False):
                """PSUM -> SBUF full-row tile, halves on parallel engines,
                then the output DMA(s)."""
                zh = z_pool.tile([P, C], out_mydt, name=f"z{m}", tag=f"z{m}")
                nc.vector.tensor_copy(zh[:, :N], ps_cc0[:])
                nc.scalar.copy(zh[:, N:], ps_cc1[:])
                if last:
                    store_row(zh, m, 0, N, eng=nc.sync)
                    store_row(zh, m, N, C, eng=nc.scalar)
                else:
                    store_row(zh, m)

            def mm(ps, k, m, cc, start, stop):
                tile_k = xwk[k]
                nc.tensor.matmul(
                    ps[:],
                    tile_k[:, m * P : (m + 1) * P],
                    tile_k[
                        :,
                        ROWS_PER_CORE + cc * N : ROWS_PER_CORE + (cc + 1) * N,
                    ],
                    start=start,
                    stop=stop,
                )

            # Group A (rows m0-m2, both column halves, 6 PSUM banks):
            # k-major in lockstep with the input stream; its 1.3us-per-k
            # consumption keeps up with the ~1.4us-per-k arrivals, so the
            # first output rows hit HBM right after the last k-tile lands.
            GA = [0, 1, 2]
            psA = {
                (m, cc): psum_pool.tile(
                    [P, N], mybir.dt.float32, name=f"psA{m}_{cc}", tag="ps"
                )
                for m in GA
                for cc in range(2)
            }
            # Warmup matmuls into psA[0,0]; the real k0 matmul below has
            # start=True, which clears the bank, so these leave no trace.
            for _ in range(14):
                nc.tensor.matmul(psA[(0, 0)][:, :P], wu[:, :P], wu[:, P:])
            for k in range(K_T):
                for m in GA:
                    for cc in range(2):
                        mm(psA[(m, cc)], k, m, cc, k == 0, k == K_T - 1)
            for m in GA:
                evict_row(psA[(m, 0)], psA[(m, 1)], m)

            # Groups B/C (rows m3-m7): m-major - each row tile completes
            # ~3.4us after the previous and streams out immediately, keeping
            # the output DMA engines saturated to the end.  The final tile's
            # two half-row stores go on parallel engines to shorten the tail.
            for m in range(3, M_T):
                ps0 = psum_pool.tile([P, N], mybir.dt.float32, name=f"psB{m}_0", tag="ps")
                ps1 = psum_pool.tile([P, N], mybir.dt.float32, name=f"psB{m}_1", tag="ps")
                for k in range(K_T):
                    mm(ps0, k, m, 0, k == 0, k == K_T - 1)
                    mm(ps1, k, m, 1, k == 0, k == K_T - 1)
                evict_row(ps0, ps1, m, last=(m == M_T - 1))

    _split_multiwaits(nc)
    return nc


_NC_CACHE: dict = {}


def _get_nc(compute_dt: str, out_dt: str, rep_mode: str) -> bass.Bass:
    key = (compute_dt, out_dt, rep_mode)
    if key not in _NC_CACHE:
        _NC_CACHE[key] = _build(compute_dt, out_dt, rep_mode)
    return _NC_CACHE[key]


def kernel(x_q, x_kv, Wq, Wk, Wv, Wproj, _compute_dt=None, _out_dt=None):
    compute_dt = _compute_dt or COMPUTE_DT
    out_dt = _out_dt or OUT_DT
    B, Tkv, C_ = x_kv.shape
    assert (B, Tkv, C_) == (4, 2048, C)

    # Fold the two projections: z = x @ Wv.T @ Wproj.T = x @ WfT
    WfT = (Wv.astype(np.float64).T @ Wproj.astype(np.float64).T).astype(np.float32)

    x_flat = x_kv.reshape(B * Tkv, C)
    in_maps = []
    for c in range(N_CORES):
        shard = x_flat[c * ROWS_PER_CORE : (c + 1) * ROWS_PER_CORE]
        xw = np.concatenate([shard.T, WfT], axis=1)  # [C(k), 2048]
        if compute_dt == "bf16":
            import ml_dtypes

            xw = xw.astype(ml_dtypes.bfloat16)
        else:
            xw = np.ascontiguousarray(xw)
        in_maps.append({"xw": xw})

    nc = _get_nc(compute_dt, out_dt, REP_MODE)
    res = run_bass_kernel_spmd(nc, in_maps, core_ids=list(range(N_CORES)))

    Tq = L * Tkv
    blocks = [res.results[c]["out"] for c in range(N_CORES)]
    out_flat = np.concatenate(blocks, axis=0)  # [B*Tq, C]
    return out_flat.reshape(B, Tq, C).astype(np.float32)
